# revision 3
# baseline (speedup 1.0000x reference)
"""AnyPrecisionLinear (4-bit LUT-quantized linear) on 8 TRN2 NeuronCores.

Reference computes:  out = x @ W.T,  W[o,i] = lut[o, qweight[o,i]]
  x: [64, 8192] fp16, qweight: [8192, 8192] int32 (values 0..15),
  lut: [8192, 16] fp16  ->  out: [64, 8192] fp16

Strategy (tensor-parallel along out_features, per the sharding hint):
  * Host re-encodes the quantized weights for shipping: each row's 16-entry
    fp16 LUT is affine-quantized to uint8 codes (scale/offset per row), and
    the per-element weight codes are gathered so each device receives a
    [IN, 1024] uint8 code shard (1 byte/weight instead of 4).
  * Each of the 8 cores: DMA-streams its code shard, dequantizes on-device
    (uint8 -> fp16 cast on DVE/ACT, per-row affine applied in the epilogue),
    and runs the matmul on the TensorEngine against stationary x.T,
    accumulating in PSUM over the 8192-deep contraction.
  * Epilogue applies out = s[o] * acc + min[o] * sum_i(x[b,i]) and writes the
    core's [64, 1024] output shard; the host concatenates shards.
"""

import numpy as np

import concourse.bass as bass
import concourse.tile as tile
from concourse import bacc, mybir
from concourse.bass_utils import run_bass_kernel_spmd

B, IN, OUT, NCORES = 64, 8192, 8192, 8
OSH = OUT // NCORES          # 1024 output columns per core
KT = IN // 128               # 64 contraction tiles of 128
G = 8                        # DMA groups
JPG = KT // G                # 8 k-tiles per group

_cached_nc = None
_last_in_maps = None


def _build():
    global _cached_nc
    if _cached_nc is not None:
        return _cached_nc

    nc = bacc.Bacc(
        "TRN2",
        target_bir_lowering=False,
        debug=False,
        enable_asserts=False,
        num_devices=NCORES,
    )
    xT = nc.dram_tensor("xT", [KT, 128, B], mybir.dt.float16, kind="ExternalInput").ap()
    w8 = nc.dram_tensor(
        "w8", [G, 128, JPG * OSH], mybir.dt.uint8, kind="ExternalInput"
    ).ap()
    sb = nc.dram_tensor("sb", [B, OSH], mybir.dt.float32, kind="ExternalInput").ap()
    mx = nc.dram_tensor("mx", [B, OSH], mybir.dt.float32, kind="ExternalInput").ap()
    out = nc.dram_tensor("out", [B, OSH], mybir.dt.float16, kind="ExternalOutput").ap()

    with tile.TileContext(nc) as tc:
        with (
            tc.tile_pool(name="xp", bufs=1) as xpool,
            tc.tile_pool(name="wp", bufs=3) as wpool,
            tc.tile_pool(name="fp", bufs=3) as fpool,
            tc.tile_pool(name="pp", bufs=1, space="PSUM") as ppool,
            tc.tile_pool(name="ep", bufs=1) as epool,
        ):
            # Stationary x.T: all 64 k-tiles side by side in the free dim.
            xt = xpool.tile([128, KT * B], mybir.dt.float16)
            nc.sync.dma_start(
                xt[:].rearrange("p (k b) -> p k b", k=KT),
                xT.rearrange("k p b -> p k b"),
            )
            sbt = epool.tile([B, OSH], mybir.dt.float32)
            nc.sync.dma_start(sbt[:], sb)
            mxt = epool.tile([B, OSH], mybir.dt.float32)
            nc.sync.dma_start(mxt[:], mx)

            psA = ppool.tile([B, 512], mybir.dt.float32)
            psB = ppool.tile([B, 512], mybir.dt.float32)

            for g in range(G):
                w8t = wpool.tile([128, JPG * OSH], mybir.dt.uint8)
                nc.sync.dma_start(w8t[:], w8[g])
                wf = fpool.tile([128, JPG * OSH], mybir.dt.float16)
                # Split the uint8->fp16 dequant cast across DVE and ACT so
                # neither becomes the pipeline bottleneck.
                if g in (1, 4, 7):
                    nc.scalar.copy(wf[:], w8t[:])
                else:
                    nc.vector.tensor_copy(wf[:], w8t[:])
                for j in range(JPG):
                    k = g * JPG + j
                    lhsT = xt[:, k * B : (k + 1) * B]
                    rhs = wf[:, j * OSH : (j + 1) * OSH]
                    nc.tensor.matmul(
                        psA[:],
                        lhsT,
                        rhs[:, 0:512],
                        start=(k == 0),
                        stop=(k == KT - 1),
                    )
                    nc.tensor.matmul(
                        psB[:],
                        lhsT,
                        rhs[:, 512:1024],
                        start=(k == 0),
                        stop=(k == KT - 1),
                    )

            t1 = epool.tile([B, OSH], mybir.dt.float32)
            o16 = epool.tile([B, OSH], mybir.dt.float16)
            nc.vector.tensor_mul(t1[:, 0:512], psA[:], sbt[:, 0:512])
            nc.vector.tensor_mul(t1[:, 512:OSH], psB[:], sbt[:, 512:OSH])
            nc.vector.tensor_add(o16[:, 0:512], t1[:, 0:512], mxt[:, 0:512])
            nc.vector.tensor_add(o16[:, 512:OSH], t1[:, 512:OSH], mxt[:, 512:OSH])
            nc.sync.dma_start(out, o16[:])

    nc.compile()
    _cached_nc = nc
    return nc


def kernel(x, qweight, lut):
    x = np.asarray(x, dtype=np.float16)
    qweight = np.asarray(qweight, dtype=np.int32)
    lut = np.asarray(lut, dtype=np.float16)

    # Per-row affine re-encode of the LUT into uint8 codes.
    lut32 = lut.astype(np.float32)
    mn = lut32.min(axis=1)
    mx_ = lut32.max(axis=1)
    rng = mx_ - mn
    rng[rng == 0] = 1.0
    s = (rng / 255.0).astype(np.float32)               # [OUT]
    lutcodes = np.rint((lut32 - mn[:, None]) * (255.0 / rng)[:, None]).astype(np.uint8)

    # Per-element weight codes, then shard + transpose for shipping.
    codes = np.take_along_axis(lutcodes, qweight, axis=1)  # [OUT, IN] uint8

    xT = np.ascontiguousarray(x.T).reshape(KT, 128, B)     # [64,128,64] fp16
    xsum = x.astype(np.float32).sum(axis=1)                # [B]

    in_maps = []
    for c in range(NCORES):
        sl = slice(c * OSH, (c + 1) * OSH)
        # [IN, OSH] codes laid out as [G, 128, JPG*OSH]: group g, partition p
        # holds its JPG k-tile rows contiguously.
        wt = codes[sl, :].T                                # [IN, OSH] view
        w8c = np.ascontiguousarray(
            wt.reshape(G, JPG, 128, OSH).transpose(0, 2, 1, 3)
        ).reshape(G, 128, JPG * OSH)
        sbc = np.ascontiguousarray(np.broadcast_to(s[sl], (B, OSH)))
        mxc = np.ascontiguousarray(np.outer(xsum, mn[sl]).astype(np.float32))
        in_maps.append({"xT": xT, "w8": w8c, "sb": sbc, "mx": mxc})

    global _last_in_maps
    _last_in_maps = in_maps

    nc = _build()
    res = run_bass_kernel_spmd(nc, in_maps, core_ids=list(range(NCORES)))
    return np.concatenate(
        [res.results[c]["out"] for c in range(NCORES)], axis=1
    ).astype(np.float16)


# revision 13
# speedup vs baseline: 1.3369x; 1.3369x over previous
"""AnyPrecisionLinear (4-bit LUT-quantized linear) on 8 TRN2 NeuronCores.

Reference computes:  out = x @ W.T,  W[o,i] = lut[o, qweight[o,i]]
  x: [64, 8192] fp16, qweight: [8192, 8192] int32 (values 0..15),
  lut: [8192, 16] fp16  ->  out: [64, 8192] fp16

Strategy (tensor-parallel along out_features, per the sharding hint):
  * Host re-encodes the quantized weights for shipping: each row's 16-entry
    fp16 LUT is affine-quantized to uint8 codes (scale s[o], offset mn[o]),
    and the per-element weight codes are gathered so each device receives a
    [8192, 1024] uint8 code shard (1 byte/weight instead of 4).
  * Each core: DMA-streams its code shard, dequantizes on-device
    (uint8 -> fp16 cast split across DVE and ACT, per-row affine applied via
    the epilogue scale + a rank-1 matmul fold), and accumulates
    x @ codes.T on the TensorEngine in PSUM over the 8192-deep contraction.
    The two 512-column halves run as concurrent column-tiled matmuls
    (output partitions 0-63 and 64-127), doubling PE throughput at M=64.
  * The mn[o]*sum_i(x[b,i]) dequant term is folded into the matmul as one
    extra contraction tile (row0 = xsum/16 against row0 = 16*mn).
  * Epilogue: out = psum * s[o] (one DVE op), DMA the [64,1024] shard out;
    the host concatenates shards.
"""

import numpy as np

import concourse.bass as bass
import concourse.tile as tile
from concourse import bacc, mybir
from concourse.bass_utils import run_bass_kernel_spmd

B, IN, OUT, NCORES = 64, 8192, 8192, 8
OSH = OUT // NCORES          # 1024 output columns per core
KT = IN // 128               # 64 contraction tiles of 128
G = 8                        # cast/DMA groups
JPG = KT // G                # 8 k-tiles per group
ACT_GROUPS = (1, 3, 5)       # groups cast on the Scalar engine (rest on DVE)
WARMUP_MMS = 120             # tiny matmuls to lift the PE HAM throttle

# Feature flags (for bisection/tuning)
USE_WARMUP = True
USE_COLTILE = True

_cached_nc = None
_last_in_maps = None


def _build():
    global _cached_nc
    if _cached_nc is not None:
        return _cached_nc

    nc = bacc.Bacc(
        "TRN2",
        target_bir_lowering=False,
        debug=False,
        enable_asserts=False,
        num_devices=NCORES,
    )
    # Host ships x.T pre-arranged as the exact SBUF image [128, (KT+1)*64]:
    # partition p, free k*64+b = x[b, k*128+p]; tile KT row0 holds xsum/16.
    xsb = nc.dram_tensor(
        "xsb", [128, (KT + 1) * B], mybir.dt.float16, kind="ExternalInput"
    ).ap()
    # Weight codes as the exact SBUF image [128, KT*OSH]:
    # partition p, free k*OSH+o = codes[o_shard, k*128+p].
    w8 = nc.dram_tensor("w8", [128, KT * OSH], mybir.dt.uint8, kind="ExternalInput").ap()
    # Rank-1 fold operand: row0 = 16*mn[o_shard], other rows zero.
    mnr = nc.dram_tensor("mnr", [128, OSH], mybir.dt.float16, kind="ExternalInput").ap()
    # Per-output-column scale, col-tiled broadcast: sb2[h*64+b, o'] = s[h*512+o'].
    sb2 = nc.dram_tensor("sb2", [128, 512], mybir.dt.float32, kind="ExternalInput").ap()
    out = nc.dram_tensor("out", [B, OSH], mybir.dt.float16, kind="ExternalOutput").ap()

    GSZ = JPG * OSH  # free-dim elements per group

    with tile.TileContext(nc) as tc:
        with (
            tc.tile_pool(name="xp", bufs=1) as xpool,
            tc.tile_pool(name="wp", bufs=3) as wpool,
            tc.tile_pool(name="fp", bufs=3) as fpool,
            tc.tile_pool(name="pp", bufs=1, space="PSUM") as ppool,
            tc.tile_pool(name="ep", bufs=1) as epool,
        ):
            # PE warmup: unthrottle HAM while input DMAs are in flight.
            if USE_WARMUP:
                wz = xpool.tile([128, 32], mybir.dt.float16)
                nc.vector.memset(wz[:], 0)
                wps = ppool.tile([32, 32], mybir.dt.float32)
                for _ in range(WARMUP_MMS):
                    nc.tensor.matmul(wps[:], wz[:, 0:32], wz[:], start=True, stop=True)

            # Inputs. First weight group goes first so the cast pipe starts ASAP.
            w8ts = []
            for g in range(G):
                w8t = wpool.tile([128, GSZ], mybir.dt.uint8)
                nc.sync.dma_start(w8t[:], w8[:, g * GSZ : (g + 1) * GSZ])
                w8ts.append(w8t)
                if g == 0:
                    xt = xpool.tile([128, (KT + 1) * B], mybir.dt.float16)
                    nc.sync.dma_start(xt[:], xsb)
                    mnt = epool.tile([128, OSH], mybir.dt.float16)
                    nc.sync.dma_start(mnt[:], mnr)
                    sbt = epool.tile([128, 512], mybir.dt.float32)
                    nc.sync.dma_start(sbt[:], sb2)

            # Separate PSUM banks per accumulation chain: a start=True in one
            # bank's zero region must not clobber the other chain's state.
            # Partition ranges stay aligned with the epilogue/output layout.
            ps1 = ppool.tile([128, 512], mybir.dt.float32)
            ps2 = ppool.tile([128, 512], mybir.dt.float32)
            psA = ps1[0:64, :]
            psB = ps2[64:128, :]

            # Rank-1 fold opens the accumulation group.
            xs_lhs = xt[:, KT * B : (KT + 1) * B]
            nc.tensor.matmul(psA, xs_lhs, mnt[:, 0:512], start=True, stop=False)
            nc.tensor.matmul(psB, xs_lhs, mnt[:, 512:1024], start=True, stop=False)

            for g in range(G):
                wf = fpool.tile([128, GSZ], mybir.dt.float16)
                if g in ACT_GROUPS:
                    nc.scalar.copy(wf[:], w8ts[g][:])
                else:
                    nc.vector.tensor_copy(wf[:], w8ts[g][:])
                for j in range(JPG):
                    k = g * JPG + j
                    lhsT = xt[:, k * B : (k + 1) * B]
                    rhs = wf[:, j * OSH : (j + 1) * OSH]
                    last = k == KT - 1
                    nc.tensor.matmul(
                        psA, lhsT, rhs[:, 0:512], start=False, stop=last
                    )
                    nc.tensor.matmul(
                        psB, lhsT, rhs[:, 512:1024], start=False, stop=last
                    )

            # Epilogue: per-column scale, fp16 cast on the way out.
            o16 = epool.tile([128, 512], mybir.dt.float16)
            nc.vector.tensor_mul(o16[0:64, :], psA, sbt[0:64, :])
            nc.vector.tensor_mul(o16[64:128, :], psB, sbt[64:128, :])
            nc.sync.dma_start(out[:, 0:512], o16[0:64, :])
            nc.sync.dma_start(out[:, 512:1024], o16[64:128, :])

    nc.compile()
    _cached_nc = nc
    return nc


def kernel(x, qweight, lut):
    x = np.asarray(x, dtype=np.float16)
    qweight = np.asarray(qweight, dtype=np.int32)
    lut = np.asarray(lut, dtype=np.float16)

    # Per-row affine re-encode of the LUT into uint8 codes.
    lut32 = lut.astype(np.float32)
    mn = lut32.min(axis=1)
    mx_ = lut32.max(axis=1)
    rng = mx_ - mn
    rng[rng == 0] = 1.0
    s = (rng / 255.0).astype(np.float32)               # [OUT]
    lutcodes = np.rint((lut32 - mn[:, None]) * (255.0 / rng)[:, None]).astype(np.uint8)

    # Per-element weight codes.
    codes = np.take_along_axis(lutcodes, qweight, axis=1)  # [OUT, IN] uint8

    # x SBUF image + xsum fold row.
    xsum = x.astype(np.float32).sum(axis=1)                # [B]
    xsb = np.zeros((128, (KT + 1) * B), np.float16)
    xsb[:, : KT * B] = (
        np.ascontiguousarray(x.T).reshape(KT, 128, B).transpose(1, 0, 2).reshape(128, KT * B)
    )
    xsb[0, KT * B :] = (xsum / 16.0).astype(np.float16)

    in_maps = []
    for c in range(NCORES):
        sl = slice(c * OSH, (c + 1) * OSH)
        wt = codes[sl, :].T                                # [IN, OSH] view
        w8c = np.ascontiguousarray(
            wt.reshape(KT, 128, OSH).transpose(1, 0, 2)
        ).reshape(128, KT * OSH)
        # Fold row carries mn/s so the epilogue's *s recovers s*acc + mn*xsum.
        mnc = np.zeros((128, OSH), np.float16)
        mnc[0, :] = (mn[sl] / s[sl] * 16.0).astype(np.float16)
        sc = s[sl]
        sb2 = np.ascontiguousarray(
            np.broadcast_to(sc.reshape(2, 512)[:, None, :], (2, B, 512)).reshape(128, 512)
        )
        in_maps.append({"xsb": xsb, "w8": w8c, "mnr": mnc, "sb2": sb2})

    global _last_in_maps
    _last_in_maps = in_maps

    nc = _build()
    res = run_bass_kernel_spmd(nc, in_maps, core_ids=list(range(NCORES)))
    return np.concatenate(
        [res.results[c]["out"] for c in range(NCORES)], axis=1
    ).astype(np.float16)
